# revision 1
# baseline (speedup 1.0000x reference)
"""Chamfer-distance (CDLoss) Trainium2 Bass kernel — single-pass, batched.

Problem: srcs, tgts [B=8, D=3, N=4096] fp32.
  P[b,i,j] = |s_i|^2 + |t_j|^2 - 2 s_i.t_j
  out = min(P, axis=1).mean() + min(P, axis=2).mean()   (scalar fp32)

Strategy (data-parallel over B across 8 NeuronCores, one batch per core):
  Each 128x4096 distance tile is computed ONCE (TensorE, bf16 hi/lo-split
  features, K=18) and consumed for BOTH reductions.  Column halves
  (2048) are the unit: TensorE -> PSUM f32 -> ScalarE cast to fp16 into
  a "ring" (4 tiles per ring, 2 rings per 8-unit block).

  All VectorE min work runs in fp16 2x_1P mode (measured: TT = 148 +
  w/2 cyc; tensor_reduce is always 1x = 141 + w cyc; scans/TTR/Max8 are
  slow or crash), and is BATCHED across 8 units with 3D APs to amortize
  instruction overhead:
    ROW (per-source mins): fold 2048->1024 per 4 units, then batched
      folds 1024->...->128 across 8 units, one batched 1x reduce
      [128,8,128] -> Rm[:, 8].
    COL (per-target mins): pair-tree across the 8 cast tiles
      (8->4->2->1) then one accumulate into A[128, 2048].
  Half finale: XBAR DMA-transpose A (DMA engines), fp16 fold + small
  reduce gives the 2048 column mins; overlaps the other half's compute.

  Per-core outputs: out_r [128, 64] f32 (row mins; col m+32*h, row p =
  source m*128+p) and out_c [128, 32] f16 (col mins, order-free);
  host does the means.
"""

import numpy as np
import ml_dtypes

_BF16 = ml_dtypes.bfloat16

# Problem geometry (hardcoded per contest contract).
_B = 8
_D = 3
_N = 4096
_P = 128              # partitions / queries per M-tile
_K = 18               # feature rows (see _features)
_NCORES = 8
_CHUNK = 2048         # PSUM chunk columns (4 banks) == column half

_prog_cache = {}

# test-harness knobs (the grading harness just calls kernel() and never
# touches these; default is the fast no-trace path)
TRACE = False
TRACE_CORES = [0]
LAST_RESULTS = None


def _build_program(n_pts=_N):
    import concourse.mybir as mybir
    import concourse.tile as tile
    from concourse import bacc

    P = _P
    MT = n_pts // P              # 32 m-tiles of 128 sources
    K = _K
    NH = n_pts // _CHUNK         # 2 column halves
    NB = _CHUNK // P             # 16 transpose blocks per half
    NBLK = MT // 8               # 4 eight-unit blocks per half
    f32 = mybir.dt.float32
    f16 = mybir.dt.float16
    bf16 = mybir.dt.bfloat16
    MIN = mybir.AluOpType.min
    AX = mybir.AxisListType.X

    nc = bacc.Bacc("TRN2", target_bir_lowering=False, debug=False,
                   num_devices=_NCORES)

    dram_w = nc.dram_tensor("w", [K, n_pts], bf16, kind="ExternalInput")
    dram_r = nc.dram_tensor("r", [K, n_pts], bf16, kind="ExternalInput")
    dram_out_r = nc.dram_tensor("out_r", [P, NH * MT], f32,
                                kind="ExternalOutput")
    dram_out_c = nc.dram_tensor("out_c", [P, NH * NB], f16,
                                kind="ExternalOutput")

    with tile.TileContext(nc) as tc:
        with (
            tc.tile_pool(name="const", bufs=1) as cpool,
            tc.tile_pool(name="ring", bufs=2) as ringpool,
            tc.tile_pool(name="stage", bufs=1) as spool,
            tc.tile_pool(name="acc", bufs=2) as apool,
            tc.tile_pool(name="trans", bufs=2) as tpool,
            tc.tile_pool(name="outs", bufs=2) as rpool,
            tc.tile_pool(name="psum", bufs=2, space="PSUM") as ppool,
        ):
            # Prologue: feature loads replicated into 4 PE row groups so
            # consecutive matmuls rotate groups (LDWEIGHTS overlaps
            # MATMUL).  Two HWDGE queues; neither is the ScalarE queue
            # (ScalarE is ~fully busy casting).
            sbW = cpool.tile([128, n_pts], bf16, tag="sbW", name="sbW")
            sbR = cpool.tile([128, n_pts], bf16, tag="sbR", name="sbR")
            # All loads on the sync queue, W/R interleaved per group, so
            # group g's pair lands early and the ScalarE queue stays free
            # for the first cast (its queue is strict FIFO).  Two small
            # priority slices cover exactly unit 0's operands (m-tile 0
            # weights + first column half) so its matmuls start ~2us
            # earlier; the group-0 bulk loads exclude that range to keep
            # the dependency on the priority DMAs only.
            nc.sync.dma_start(sbW[0:K, 0:P], dram_w[:, 0:P])
            nc.sync.dma_start(sbR[0:K, 0:512], dram_r[:, 0:512])
            nc.sync.dma_start(sbR[0:K, 512:_CHUNK], dram_r[:, 512:_CHUNK])
            # group 1 rides the otherwise-idle ScalarE queue in parallel
            # (it drains well before the first cast is ready to issue)
            nc.scalar.dma_start(sbW[32:32 + K, :], dram_w[:])
            nc.scalar.dma_start(sbR[32:32 + K, :], dram_r[:])
            for g in range(2, 4):
                nc.sync.dma_start(sbW[32 * g:32 * g + K, :], dram_w[:])
                nc.sync.dma_start(sbR[32 * g:32 * g + K, :], dram_r[:])
            # group-0 bulk (first needed by unit 4) goes last
            nc.sync.dma_start(sbW[0:K, P:], dram_w[:, P:])
            nc.sync.dma_start(sbR[0:K, _CHUNK:], dram_r[:, _CHUNK:])

            V = nc.vector
            for h in range(NH):
                A = apool.tile([P, _CHUNK], f16, tag="A", name=f"A{h}")
                # cross-block staging: per-block COL results and ROW
                # fold-to-256 partials, combined once per half
                CC = spool.tile([P, NBLK, _CHUNK], f16, tag="CC")
                SS = spool.tile([P, NBLK, 8, 256], f16, tag="SS")
                for blk in range(NBLK):
                    rings = [
                        ringpool.tile([P, 4, _CHUNK], f16, tag=f"ring{i}",
                                      name=f"ring{i}")
                        for i in range(2)
                    ]
                    S1 = spool.tile([P, 8, 1024], f16, tag="S1")
                    C2 = spool.tile([P, 4, _CHUNK], f16, tag="C2")
                    first = h == 0 and blk == 0
                    for j in range(8):
                        m = blk * 8 + j
                        ring, jj = rings[j // 4], j % 4
                        ps = ppool.tile([P, _CHUNK], f32, tag="ps")
                        for q in range(4):
                            # row group rotates per 512-col matmul so
                            # LDWEIGHTS overlaps in-flight MATMULs.  The
                            # first 4 units pin all their matmuls to one
                            # group so unit u depends only on the u-th
                            # prologue DMA pair (shorter kernel-start
                            # ramp); PE has large slack there.
                            g = j if first and j < 4 else q
                            col = _CHUNK * h + 512 * q
                            nc.tensor.matmul(
                                ps[:, 512 * q:512 * (q + 1)],
                                sbW[32 * g:32 * g + K, m * P:(m + 1) * P],
                                sbR[32 * g:32 * g + K, col:col + 512],
                                start=True, stop=True,
                                tile_position=(32 * g, 0),
                            )
                        nc.scalar.copy(ring[:, jj, :], ps[:])
                        if first and j % 2 == 1:
                            # kernel-start ramp: sub-batch by 2 so VectorE
                            # starts two casts earlier
                            half, sub = j // 4, (j // 2) % 2
                            V.tensor_tensor(
                                S1[:, j - 1:j + 1, :],
                                ring[:, 2 * sub:2 * sub + 2, 0:1024],
                                ring[:, 2 * sub:2 * sub + 2, 1024:2048],
                                op=MIN)
                            V.tensor_tensor(
                                C2[:, j // 2:j // 2 + 1, :],
                                ring[:, 2 * sub:2 * sub + 1, :],
                                ring[:, 2 * sub + 1:2 * sub + 2, :],
                                op=MIN)
                        elif not first and (j == 3 or j == 7):
                            half = j // 4
                            # ROW fold level 1 for these 4 units
                            V.tensor_tensor(
                                S1[:, 4 * half:4 * half + 4, :],
                                ring[:, :, 0:1024], ring[:, :, 1024:2048],
                                op=MIN)
                            # COL pair tree level 1: (0,1),(2,3)
                            V.tensor_tensor(
                                C2[:, 2 * half:2 * half + 2, :],
                                ring[:, 0:4:2, :], ring[:, 1:4:2, :],
                                op=MIN)
                    # block tail: COL tree merge into CC, ROW fold into SS
                    C4 = spool.tile([P, 2, _CHUNK], f16, tag="C4")
                    V.tensor_tensor(C4[:], C2[:, 0:4:2, :],
                                    C2[:, 1:4:2, :], op=MIN)
                    V.tensor_tensor(CC[:, blk, :], C4[:, 0, :],
                                    C4[:, 1, :], op=MIN)
                    S2 = spool.tile([P, 8, 512], f16, tag="S2")
                    V.tensor_tensor(S2[:], S1[:, :, 0:512],
                                    S1[:, :, 512:1024], op=MIN)
                    V.tensor_tensor(SS[:, blk, :, :], S2[:, :, 0:256],
                                    S2[:, :, 256:512], op=MIN)
                # Half finale.  COL: combine per-block results, then
                # XBAR-transpose A so column mins become a free-axis
                # fold+reduce (DMA engines move it while VectorE runs
                # the ROW tail).
                D1 = spool.tile([P, 2, _CHUNK], f16, tag="D1")
                V.tensor_tensor(D1[:], CC[:, 0:2, :], CC[:, 2:4, :],
                                op=MIN)
                V.tensor_tensor(A[:], D1[:, 0, :], D1[:, 1, :], op=MIN)
                TA = tpool.tile([P, NB, P], f16, tag="TA", name=f"TA{h}")
                nc.sync.dma_start_transpose(TA[:], A[:])
                T4 = spool.tile([P, NBLK, 8, 128], f16, tag="T4")
                V.tensor_tensor(T4[:], SS[:, :, :, 0:128],
                                SS[:, :, :, 128:256], op=MIN)
                T5 = spool.tile([P, NBLK, 8, 64], f16, tag="T5")
                V.tensor_tensor(T5[:], T4[:, :, :, 0:64],
                                T4[:, :, :, 64:128], op=MIN)
                T6 = spool.tile([P, NBLK, 8, 32], f16, tag="T6")
                V.tensor_tensor(T6[:], T5[:, :, :, 0:32],
                                T5[:, :, :, 32:64], op=MIN)
                Rm = rpool.tile([P, MT], f32, tag="Rm", name=f"Rm{h}")
                V.tensor_reduce(Rm[:], T6[:], axis=AX, op=MIN)
                F1 = spool.tile([P, NB, 64], f16, tag="F1")
                V.tensor_tensor(F1[:], TA[:, :, 0:64], TA[:, :, 64:128],
                                op=MIN)
                F2 = spool.tile([P, NB, 32], f16, tag="F2")
                V.tensor_tensor(F2[:], F1[:, :, 0:32], F1[:, :, 32:64],
                                op=MIN)
                C = rpool.tile([P, NB], f16, tag="C", name=f"C{h}")
                V.tensor_reduce(C[:], F2[:], axis=AX, op=MIN)
                nc.sync.dma_start(
                    dram_out_r[:, h * MT:(h + 1) * MT], Rm[:])
                nc.scalar.dma_start(
                    dram_out_c[:, h * NB:(h + 1) * NB], C[:])

    nc.compile()
    return nc


def _get_program(n_pts=_N):
    if n_pts not in _prog_cache:
        _prog_cache[n_pts] = _build_program(n_pts)
    return _prog_cache[n_pts]


def _split_bf16(x32):
    """x32 fp32 -> (hi, lo) bf16 with hi+lo ~= x to ~2^-18 rel."""
    hi = x32.astype(_BF16)
    lo = (x32 - hi.astype(np.float32)).astype(_BF16)
    return hi, lo


def _split3(x64):
    """fp64 vector -> 3 bf16 terms summing to x to ~2^-27 rel."""
    t0 = x64.astype(_BF16)
    r = x64 - t0.astype(np.float64)
    t1 = r.astype(_BF16)
    r2 = r - t1.astype(np.float64)
    t2 = r2.astype(_BF16)
    return t0, t1, t2


def _features(q, c, n_pts):
    """Feature tensors for the distance matmul.

    q: query points  [3, N] fp32; c: candidate points [3, N] fp32.
    Returns (W [18, N] bf16, R [18, N] bf16) with
      (W.T @ R)[i, j] ~= |q~_i - c~_j|^2
    with ~ the bf16-split (hi+lo) values, exact to ~2e-6.
    """
    q_hi, q_lo = _split_bf16(q)
    c_hi, c_lo = _split_bf16(c)
    q_t = q_hi.astype(np.float32) + q_lo.astype(np.float32)
    c_t = c_hi.astype(np.float32) + c_lo.astype(np.float32)

    U = (c_t.astype(np.float64) ** 2).sum(axis=0)   # candidate norms
    u0, u1, u2 = _split3(U)
    V = (q_t.astype(np.float64) ** 2).sum(axis=0)   # query norms
    v0, v1, v2 = _split3(V)

    m2q_hi = (-2.0 * q_hi.astype(np.float32)).astype(_BF16)
    m2q_lo = (-2.0 * q_lo.astype(np.float32)).astype(_BF16)
    ones = np.ones(n_pts, dtype=_BF16)

    Wg = np.concatenate([
        m2q_hi, m2q_hi, m2q_lo, m2q_lo,
        np.stack([ones, ones, ones]),
        np.stack([v0, v1, v2]),
    ], axis=0).astype(_BF16)              # [18, N]
    Rg = np.concatenate([
        c_hi, c_lo, c_hi, c_lo,
        np.stack([u0, u1, u2]),
        np.stack([ones, ones, ones]),
    ], axis=0).astype(_BF16)              # [18, N]

    return Wg, Rg


def kernel(srcs, tgts):
    import concourse.bass_utils as bass_utils

    srcs = np.asarray(srcs, dtype=np.float32)
    tgts = np.asarray(tgts, dtype=np.float32)
    B = srcs.shape[0]
    assert srcs.shape == (B, _D, _N) and tgts.shape == (B, _D, _N)

    nc = _get_program()

    in_maps = []
    for b in range(B):
        W, R = _features(srcs[b], tgts[b], _N)  # queries = sources
        in_maps.append({"w": W, "r": R})

    res = None
    for attempt in range(3):
        try:
            res = bass_utils.run_bass_kernel_spmd(
                nc, in_maps, core_ids=list(range(_NCORES)),
                trace=TRACE, trace_cores=TRACE_CORES if TRACE else None,
            )
            break
        except Exception:
            # transient NRT/device hiccups have been observed; retry
            if attempt == 2:
                raise
            import time
            time.sleep(3.0)
    global LAST_RESULTS
    LAST_RESULTS = res

    total = 0.0
    for b in range(B):
        out_r = res.results[b]["out_r"]   # [128, 64] f32 per-source mins
        out_c = res.results[b]["out_c"]   # [128, 32] f16 per-target mins
        row = np.minimum(out_r[:, :32], out_r[:, 32:]).astype(np.float64)
        col = out_c.astype(np.float64)
        # reference: min(P, axis=1).mean() -> per-target mins (col);
        #            min(P, axis=2).mean() -> per-source mins (row)
        total += col.mean() + row.mean()

    return np.float32(total / B)



# revision 3
# speedup vs baseline: 1.1156x; 1.1156x over previous
"""Chamfer-distance (CDLoss) Trainium2 Bass kernel — exp-remap softmin hybrid.

Problem: srcs, tgts [B=8, D=3, N=4096] fp32.
  P[b,i,j] = |s_i|^2 + |t_j|^2 - 2 s_i.t_j
  out = min(P, axis=1).mean() + min(P, axis=2).mean()   (scalar fp32)

Strategy (data-parallel over B across 8 NeuronCores, one batch per core):
  The 4096x4096 distance matrix is produced tile-by-tile on TensorE
  (bf16 hi/lo-split features, K=18, [128,2048]-column units, PSUM f32)
  exactly as the classical baseline.  The bottleneck engine used to be
  VectorE (both min-trees, ~147us); the redesign moves the row
  reduction into the Activation engine's PSUM drain:

  * Act applies E = exp((beta - P)/T) (bias=beta/T per-partition const,
    scale=-1/T) while draining PSUM -> SBUF bf16.  Cost identical to
    the plain cast (measured 1967ns/tile), and `accum_out` yields the
    per-partition row sums Sum_j E for free (+182ns/tile): the row
    softmin is beta - T*ln(sum), computed on host.  Row min-tree: gone.
  * The col direction stays a classical tree, but in E-space: E is a
    monotone remap of P, so col max of E == col min of P (exact to
    bf16 rounding).  VectorE pair-max tree over the 7 exp'd tiles per
    block -> CC -> D1 -> A; A [128,2048] per half is shipped to DRAM
    and the host does the final 128-partition max (no device transpose
    tail).
  * 8 "direct" units (m%8==7, both halves) skip Act entirely: VectorE
    drains their PSUM with a 32:1 tensor_reduce min (rows, exact f16)
    and a mixed f32/f16 TT min into a raw per-half col accumulator.
    This rebalances ~17us of Act work onto VectorE's slack.

  Numerics: T=1e-3.  exp underflow flushes far pairs (harmless for
  min); rows/cols whose true min exceeds beta+~85T can flush/overflow
  - they are detected on host (nonfinite / threshold) and recomputed
  exactly there (~160 rows + ~160 cols per batch, measured; host cost
  ~60 Mflops).  Simulated end-to-end rel err 3.3e-3 vs 2e-2 gate.

  Per-core outputs:
    rs_acc [128, 64] f32   row exp-sums per (h,m) unit (exp'd units)
    rdir   [128, 8]  f16   exact row mins of the direct units
    colE   [128,4096] bf16 per-half col-max-of-E partials (over 128
                           partitions each; host maxes + log-maps)
    colR   [128,4096] f16  per-half raw col mins over direct units
"""

import numpy as np
import ml_dtypes

_BF16 = ml_dtypes.bfloat16

# Problem geometry (hardcoded per contest contract).
_B = 8
_D = 3
_N = 4096
_P = 128              # partitions / sources per m-tile
_K = 18               # feature rows (see _features)
_NCORES = 8
_CHUNK = 2048         # PSUM chunk columns (4 banks) == column half

_T = 1.0e-3           # softmin temperature
_SCALE = -1.0 / _T
_ROW_THR = 85.0 * _T  # host fallback threshold above beta
_COL_THR = 80.0 * _T

_prog_cache = {}

# test-harness knobs (the grading harness just calls kernel() and never
# touches these; default is the fast no-trace path)
TRACE = False
TRACE_CORES = [0]
LAST_RESULTS = None


def _build_program(n_pts=_N):
    import concourse.mybir as mybir
    import concourse.tile as tile
    from concourse import bacc

    P = _P
    MT = n_pts // P              # 32 m-tiles of 128 sources
    K = _K
    NH = n_pts // _CHUNK         # 2 column halves
    NBLK = MT // 8               # 4 eight-unit blocks per half
    f32 = mybir.dt.float32
    f16 = mybir.dt.float16
    bf16 = mybir.dt.bfloat16
    MIN = mybir.AluOpType.min
    MAX = mybir.AluOpType.max
    AX = mybir.AxisListType.X
    EXP = mybir.ActivationFunctionType.Exp

    nc = bacc.Bacc("TRN2", target_bir_lowering=False, debug=False,
                   num_devices=_NCORES)

    dram_w = nc.dram_tensor("w", [K, n_pts], bf16, kind="ExternalInput")
    dram_r = nc.dram_tensor("r", [K, n_pts], bf16, kind="ExternalInput")
    dram_bv = nc.dram_tensor("bv", [P, 1], f32, kind="ExternalInput")
    dram_rs = nc.dram_tensor("rs_acc", [P, NH * MT], f32,
                             kind="ExternalOutput")
    dram_rd = nc.dram_tensor("rdir", [P, 8], f16, kind="ExternalOutput")
    dram_ce = nc.dram_tensor("colE", [P, n_pts], bf16,
                             kind="ExternalOutput")
    dram_cr = nc.dram_tensor("colR", [P, n_pts], f16,
                             kind="ExternalOutput")

    with tile.TileContext(nc) as tc:
        with (
            tc.tile_pool(name="const", bufs=1) as cpool,
            tc.tile_pool(name="ring", bufs=2) as ringpool,
            tc.tile_pool(name="stage", bufs=1) as spool,
            tc.tile_pool(name="outs", bufs=1) as rpool,
            tc.tile_pool(name="psum", bufs=2, space="PSUM") as ppool,
        ):
            # Prologue: feature loads replicated into 4 PE row groups so
            # consecutive matmuls rotate groups (LDWEIGHTS overlaps
            # MATMUL).  Priority slices cover unit 0's operands so its
            # matmuls start early; bulk loads follow.  The Act queue is
            # kept DMA-free (Act is the bottleneck engine).
            sbW = cpool.tile([128, n_pts], bf16, tag="sbW", name="sbW")
            sbR = cpool.tile([128, n_pts], bf16, tag="sbR", name="sbR")
            bv = cpool.tile([P, 1], f32, tag="bv", name="bv")
            nc.sync.dma_start(bv[:], dram_bv[:])
            nc.sync.dma_start(sbW[0:K, 0:P], dram_w[:, 0:P])
            nc.sync.dma_start(sbR[0:K, 0:512], dram_r[:, 0:512])
            nc.sync.dma_start(sbR[0:K, 512:_CHUNK], dram_r[:, 512:_CHUNK])
            # bulk replicas ride the idle Vector/GpSimd DMA queues
            nc.gpsimd.dma_start(sbW[32 + 0:32 + K, :], dram_w[:])
            nc.gpsimd.dma_start(sbR[32 + 0:32 + K, :], dram_r[:])
            for g in range(2, 4):
                nc.sync.dma_start(sbW[32 * g:32 * g + K, :], dram_w[:])
                nc.sync.dma_start(sbR[32 * g:32 * g + K, :], dram_r[:])
            nc.sync.dma_start(sbW[0:K, P:], dram_w[:, P:])
            nc.sync.dma_start(sbR[0:K, _CHUNK:], dram_r[:, _CHUNK:])

            V = nc.vector
            S = nc.scalar

            RS = rpool.tile([P, NH * MT], f32, tag="RS", name="RS")
            RD = rpool.tile([P, 8], f16, tag="RD", name="RD")
            CE = rpool.tile([P, n_pts], bf16, tag="CE", name="CE")
            CR = rpool.tile([P, n_pts], f16, tag="CR", name="CR")

            for h in range(NH):
                ce_h = CE[:, h * _CHUNK:(h + 1) * _CHUNK]
                cr_h = CR[:, h * _CHUNK:(h + 1) * _CHUNK]
                CCt = spool.tile([P, NBLK, _CHUNK], bf16, tag="CC")
                for blk in range(NBLK):
                    ring0 = ringpool.tile([P, 4, _CHUNK], bf16,
                                          tag="ring0", name="ring0")
                    ring1 = ringpool.tile([P, 3, _CHUNK], bf16,
                                          tag="ring1", name="ring1")
                    C2a = spool.tile([P, 2, _CHUNK], bf16, tag="C2a")
                    C2b = spool.tile([P, _CHUNK], bf16, tag="C2b")
                    first = h == 0 and blk == 0
                    for j in range(8):
                        m = blk * 8 + j
                        u = h * MT + m
                        ps = ppool.tile([P, 64, 32], f32, tag="ps")
                        for q in range(4):
                            # row group rotates per 512-col matmul so
                            # LDWEIGHTS overlaps in-flight MATMULs.  In
                            # the very first block each unit pins one
                            # group so unit j depends only on the j-th
                            # prologue DMA pair.
                            g = j if first and j < 4 else q
                            col = _CHUNK * h + 512 * q
                            nc.tensor.matmul(
                                ps[:, 16 * q:16 * (q + 1), :],
                                sbW[32 * g:32 * g + K, m * P:(m + 1) * P],
                                sbR[32 * g:32 * g + K, col:col + 512],
                                start=True, stop=True,
                                tile_position=(32 * g, 0),
                            )
                        if j == 7:
                            # direct unit: VectorE drains PSUM.  Rows:
                            # 32:1 reduce then 64:1 (exact min, f16).
                            # Cols: mixed f32/f16 TT min into the raw
                            # per-half accumulator (copy to init).
                            diridx = h * 4 + blk
                            rd1 = spool.tile([P, 64], f16, tag="rd1")
                            V.tensor_reduce(rd1[:], ps[:], axis=AX, op=MIN)
                            V.tensor_reduce(RD[:, diridx:diridx + 1],
                                            rd1[:], axis=AX, op=MIN)
                            if blk == 0:
                                V.tensor_copy(cr_h, ps[:, :, :])
                            else:
                                V.tensor_tensor(cr_h, ps[:, :, :], cr_h,
                                                op=MIN)
                        else:
                            ring, jj = (ring0, j) if j < 4 else (ring1,
                                                                 j - 4)
                            S.activation(ring[:, jj, :], ps[:, :, :], EXP,
                                         bias=bv[:], scale=_SCALE,
                                         accum_out=RS[:, u:u + 1])
                        if j == 3:
                            V.tensor_tensor(C2a[:], ring0[:, 0:4:2, :],
                                            ring0[:, 1:4:2, :], op=MAX)
                        elif j == 6:
                            V.tensor_tensor(C2b[:], ring1[:, 0, :],
                                            ring1[:, 1, :], op=MAX)
                    # block tail: 7-tile max tree -> CC[blk]
                    C4a = spool.tile([P, _CHUNK], bf16, tag="C4a")
                    C4b = spool.tile([P, _CHUNK], bf16, tag="C4b")
                    V.tensor_tensor(C4a[:], C2a[:, 0, :], C2a[:, 1, :],
                                    op=MAX)
                    V.tensor_tensor(C4b[:], C2b[:], ring1[:, 2, :],
                                    op=MAX)
                    V.tensor_tensor(CCt[:, blk, :], C4a[:], C4b[:],
                                    op=MAX)
                # half finale: combine blocks, ship both col partials
                D1 = spool.tile([P, 2, _CHUNK], bf16, tag="D1")
                V.tensor_tensor(D1[:], CCt[:, 0:2, :], CCt[:, 2:4, :],
                                op=MAX)
                V.tensor_tensor(ce_h, D1[:, 0, :], D1[:, 1, :], op=MAX)
                nc.sync.dma_start(dram_ce[:, h * _CHUNK:(h + 1) * _CHUNK],
                                  ce_h)
                nc.sync.dma_start(dram_cr[:, h * _CHUNK:(h + 1) * _CHUNK],
                                  cr_h)
            nc.sync.dma_start(dram_rs[:], RS[:])
            nc.sync.dma_start(dram_rd[:], RD[:])

    nc.compile()
    return nc


def _get_program(n_pts=_N):
    if n_pts not in _prog_cache:
        _prog_cache[n_pts] = _build_program(n_pts)
    return _prog_cache[n_pts]


def _split_bf16(x32):
    """x32 fp32 -> (hi, lo) bf16 with hi+lo ~= x to ~2^-18 rel."""
    hi = x32.astype(_BF16)
    lo = (x32 - hi.astype(np.float32)).astype(_BF16)
    return hi, lo


def _split3(x64):
    """fp64 vector -> 3 bf16 terms summing to x to ~2^-27 rel."""
    t0 = x64.astype(_BF16)
    r = x64 - t0.astype(np.float64)
    t1 = r.astype(_BF16)
    r2 = r - t1.astype(np.float64)
    t2 = r2.astype(_BF16)
    return t0, t1, t2


def _features(q, c, n_pts):
    """Feature tensors for the distance matmul.

    q: query points  [3, N] fp32; c: candidate points [3, N] fp32.
    Returns (W [18, N] bf16, R [18, N] bf16) with
      (W.T @ R)[i, j] ~= |q~_i - c~_j|^2
    with ~ the bf16-split (hi+lo) values, exact to ~2e-6.
    """
    q_hi, q_lo = _split_bf16(q)
    c_hi, c_lo = _split_bf16(c)
    q_t = q_hi.astype(np.float32) + q_lo.astype(np.float32)
    c_t = c_hi.astype(np.float32) + c_lo.astype(np.float32)

    U = (c_t.astype(np.float64) ** 2).sum(axis=0)   # candidate norms
    u0, u1, u2 = _split3(U)
    V = (q_t.astype(np.float64) ** 2).sum(axis=0)   # query norms
    v0, v1, v2 = _split3(V)

    m2q_hi = (-2.0 * q_hi.astype(np.float32)).astype(_BF16)
    m2q_lo = (-2.0 * q_lo.astype(np.float32)).astype(_BF16)
    ones = np.ones(n_pts, dtype=_BF16)

    Wg = np.concatenate([
        m2q_hi, m2q_hi, m2q_lo, m2q_lo,
        np.stack([ones, ones, ones]),
        np.stack([v0, v1, v2]),
    ], axis=0).astype(_BF16)              # [18, N]
    Rg = np.concatenate([
        c_hi, c_lo, c_hi, c_lo,
        np.stack([u0, u1, u2]),
        np.stack([ones, ones, ones]),
    ], axis=0).astype(_BF16)              # [18, N]

    return Wg, Rg


def _host_prep(srcs_b, tgts_b):
    """Features + global softmin bias beta for one batch."""
    W, R = _features(srcs_b, tgts_b, _N)
    s = srcs_b.astype(np.float64)          # [3, N]
    t = tgts_b.astype(np.float64)
    rs = (s * s).sum(0)                    # |s_i|^2
    rt = (t * t).sum(0)
    # subsample estimate of per-row mins (stride 4 -> 1024 targets)
    tsub = t[:, ::4]
    cross = s.T @ tsub                     # [N, N/4]
    psub = rs[:, None] + rt[None, ::4] - 2.0 * cross
    bi = psub.min(axis=1)
    beta = float(bi.min())
    bvec = np.full((_P, 1), beta / _T, dtype=np.float32)
    return W, R, bvec, beta, rs, rt


def _host_post(res_b, beta, rs, rt, srcs_b, tgts_b):
    """Recover row/col minima for one batch; exact host fallback for
    flushed/overflowed entries."""
    s = srcs_b.astype(np.float64)
    t = tgts_b.astype(np.float64)
    MT = _N // _P

    rs_acc = res_b["rs_acc"].astype(np.float64)   # [128, 64]
    rdir = res_b["rdir"].astype(np.float64)       # [128, 8]
    colE = res_b["colE"].astype(np.float64)       # [128, 4096]
    colR = res_b["colR"].astype(np.float64)       # [128, 4096]

    # --- rows ---
    rowvals = np.empty(_N)
    bad_rows = []
    for m in range(MT):
        sl = slice(m * _P, (m + 1) * _P)
        if m % 8 == 7:
            d = m // 8
            rowvals[sl] = np.minimum(rdir[:, d], rdir[:, 4 + d])
        else:
            Rsum = rs_acc[:, m] + rs_acc[:, MT + m]
            with np.errstate(divide="ignore", invalid="ignore"):
                v = beta - _T * np.log(Rsum)
            bad = ~np.isfinite(v) | (v > beta + _ROW_THR)
            v = np.where(bad, np.inf, v)
            rowvals[sl] = v
            if bad.any():
                bad_rows.extend(m * _P + np.nonzero(bad)[0])
    if bad_rows:
        idx = np.asarray(bad_rows)
        d2 = (rs[idx, None] + rt[None, :]
              - 2.0 * (s[:, idx].T @ t))
        rowvals[idx] = d2.min(axis=1)

    # --- cols ---
    maxE = colE.max(axis=0)                       # [4096]
    with np.errstate(divide="ignore", invalid="ignore"):
        vE = beta - _T * np.log(maxE)
    vR = colR.min(axis=0)
    colvals = np.minimum(np.where(np.isfinite(vE), vE, np.inf), vR)
    bad_c = ~np.isfinite(colvals) | (colvals > beta + _COL_THR)
    if bad_c.any():
        jdx = np.nonzero(bad_c)[0]
        d2 = (rs[:, None] + rt[jdx][None, :]
              - 2.0 * (s.T @ t[:, jdx]))
        colvals[jdx] = d2.min(axis=0)

    return rowvals.mean() + colvals.mean()


def kernel(srcs, tgts):
    import concourse.bass_utils as bass_utils

    srcs = np.asarray(srcs, dtype=np.float32)
    tgts = np.asarray(tgts, dtype=np.float32)
    B = srcs.shape[0]
    assert srcs.shape == (B, _D, _N) and tgts.shape == (B, _D, _N)

    nc = _get_program()

    in_maps = []
    host_ctx = []
    for b in range(B):
        W, R, bvec, beta, rs, rt = _host_prep(srcs[b], tgts[b])
        in_maps.append({"w": W, "r": R, "bv": bvec})
        host_ctx.append((beta, rs, rt))

    res = None
    for attempt in range(3):
        try:
            res = bass_utils.run_bass_kernel_spmd(
                nc, in_maps, core_ids=list(range(_NCORES)),
                trace=TRACE, trace_cores=TRACE_CORES if TRACE else None,
            )
            break
        except Exception:
            # transient NRT/device hiccups have been observed; retry
            if attempt == 2:
                raise
            import time
            time.sleep(3.0)
    global LAST_RESULTS
    LAST_RESULTS = res

    total = 0.0
    for b in range(B):
        beta, rs, rt = host_ctx[b]
        total += _host_post(res.results[b], beta, rs, rt,
                            srcs[b], tgts[b])

    return np.float32(total / B)


# revision 4
# speedup vs baseline: 1.1418x; 1.0235x over previous
"""Chamfer-distance (CDLoss) Trainium2 Bass kernel — exp-remap softmin hybrid.

Problem: srcs, tgts [B=8, D=3, N=4096] fp32.
  P[b,i,j] = |s_i|^2 + |t_j|^2 - 2 s_i.t_j
  out = min(P, axis=1).mean() + min(P, axis=2).mean()   (scalar fp32)

Strategy (data-parallel over B across 8 NeuronCores, one batch per core):
  The 4096x4096 distance matrix is produced tile-by-tile on TensorE
  (bf16 hi/lo-split features, K=18, [128,2048]-column units, PSUM f32)
  exactly as the classical baseline.  The bottleneck engine used to be
  VectorE (both min-trees, ~147us); the redesign moves the row
  reduction into the Activation engine's PSUM drain:

  * Act applies E = exp((beta - P)/T) (bias=beta/T per-partition const,
    scale=-1/T) while draining PSUM -> SBUF bf16.  Cost identical to
    the plain cast (measured 1967ns/tile), and `accum_out` yields the
    per-partition row sums Sum_j E for free (+182ns/tile): the row
    softmin is beta - T*ln(sum), computed on host.  Row min-tree: gone.
  * The col direction stays a classical tree, but in E-space: E is a
    monotone remap of P, so col max of E == col min of P (exact to
    bf16 rounding).  VectorE pair-max tree over the 7 exp'd tiles per
    block -> CC -> D1 -> A; A [128,2048] per half is shipped to DRAM
    and the host does the final 128-partition max (no device transpose
    tail).
  * 8 "direct" units (m%8==7, both halves) skip Act entirely: VectorE
    drains their PSUM with a 32:1 tensor_reduce min (rows, exact f16)
    and a mixed f32/f16 TT min into a raw per-half col accumulator.
    This rebalances ~17us of Act work onto VectorE's slack.

  Numerics: T=1e-3.  exp underflow flushes far pairs (harmless for
  min); rows/cols whose true min exceeds beta+~85T can flush/overflow
  - they are detected on host (nonfinite / threshold) and recomputed
  exactly there (~160 rows + ~160 cols per batch, measured; host cost
  ~60 Mflops).  Simulated end-to-end rel err 3.3e-3 vs 2e-2 gate.

  Per-core outputs:
    rs_acc [128, 64] f32   row exp-sums per (h,m) unit (exp'd units)
    rdir   [128, 8]  f16   exact row mins of the direct units
    colE   [128,4096] bf16 per-half col-max-of-E partials (over 128
                           partitions each; host maxes + log-maps)
    colR   [128,4096] f16  per-half raw col mins over direct units
"""

import numpy as np
import ml_dtypes

_BF16 = ml_dtypes.bfloat16

# Problem geometry (hardcoded per contest contract).
_B = 8
_D = 3
_N = 4096
_P = 128              # partitions / sources per m-tile
_K = 18               # feature rows (see _features)
_NCORES = 8
_CHUNK = 2048         # PSUM chunk columns (4 banks) == column half

_T = 1.0e-3           # softmin temperature
_SCALE = -1.0 / _T
_ROW_THR = 85.0 * _T  # host fallback threshold above beta
_COL_THR = 80.0 * _T

_prog_cache = {}

# test-harness knobs (the grading harness just calls kernel() and never
# touches these; default is the fast no-trace path)
TRACE = False
TRACE_CORES = [0]
LAST_RESULTS = None


def _build_program(n_pts=_N):
    import concourse.mybir as mybir
    import concourse.tile as tile
    from concourse import bacc

    P = _P
    MT = n_pts // P              # 32 m-tiles of 128 sources
    K = _K
    NH = n_pts // _CHUNK         # 2 column halves
    NBLK = MT // 8               # 4 eight-unit blocks per half
    f32 = mybir.dt.float32
    f16 = mybir.dt.float16
    bf16 = mybir.dt.bfloat16
    MIN = mybir.AluOpType.min
    MAX = mybir.AluOpType.max
    AX = mybir.AxisListType.X
    EXP = mybir.ActivationFunctionType.Exp

    nc = bacc.Bacc("TRN2", target_bir_lowering=False, debug=False,
                   num_devices=_NCORES)

    dram_w = nc.dram_tensor("w", [K, n_pts], bf16, kind="ExternalInput")
    dram_r = nc.dram_tensor("r", [K, n_pts], bf16, kind="ExternalInput")
    dram_bv = nc.dram_tensor("bv", [P, 1], f32, kind="ExternalInput")
    dram_rs = nc.dram_tensor("rs_acc", [P, NH * MT], f32,
                             kind="ExternalOutput")
    dram_rd = nc.dram_tensor("rdir", [P, 8], f16, kind="ExternalOutput")
    dram_ce = nc.dram_tensor("colE", [P, NH * NBLK * _CHUNK], bf16,
                             kind="ExternalOutput")
    dram_cr = nc.dram_tensor("colR", [P, n_pts], f16,
                             kind="ExternalOutput")

    with tile.TileContext(nc) as tc:
        with (
            tc.tile_pool(name="const", bufs=1) as cpool,
            tc.tile_pool(name="ring", bufs=2) as ringpool,
            tc.tile_pool(name="stage", bufs=1) as spool,
            tc.tile_pool(name="outs", bufs=1) as rpool,
            tc.tile_pool(name="psum", bufs=2, space="PSUM") as ppool,
        ):
            # Prologue: feature loads replicated into 4 PE row groups so
            # consecutive matmuls rotate groups (LDWEIGHTS overlaps
            # MATMUL).  Priority slices cover unit 0's operands so its
            # matmuls start early; bulk loads follow.  The Act queue is
            # kept DMA-free (Act is the bottleneck engine).
            sbW = cpool.tile([128, n_pts], bf16, tag="sbW", name="sbW")
            sbR = cpool.tile([128, n_pts], bf16, tag="sbR", name="sbR")
            bv = cpool.tile([P, 1], f32, tag="bv", name="bv")
            warm = cpool.tile([P, 1], f32, tag="warm", name="warm")
            nc.sync.dma_start(sbW[0:K, 0:2 * P], dram_w[:, 0:2 * P])
            nc.sync.dma_start(sbR[0:K, 0:512], dram_r[:, 0:512])
            nc.sync.dma_start(sbR[0:K, 512:_CHUNK], dram_r[:, 512:_CHUNK])
            nc.sync.dma_start(bv[:], dram_bv[:])
            # warm the Act exp table at t=0 so the first real exp
            # doesn't eat the 1.3us ACT_TABLE_LOAD
            nc.vector.memset(warm[:], 0.0)
            nc.scalar.activation(warm[:], warm[:],
                                 mybir.ActivationFunctionType.Exp,
                                 bias=0.0, scale=1.0)
            # bulk replicas ride the idle GpSimd (SWDGE) queue
            nc.gpsimd.dma_start(sbW[32 + 0:32 + K, :], dram_w[:])
            nc.gpsimd.dma_start(sbR[32 + 0:32 + K, :], dram_r[:])
            for g in range(2, 4):
                nc.sync.dma_start(sbW[32 * g:32 * g + K, :], dram_w[:])
                nc.sync.dma_start(sbR[32 * g:32 * g + K, :], dram_r[:])
            nc.sync.dma_start(sbW[0:K, 2 * P:], dram_w[:, 2 * P:])
            nc.sync.dma_start(sbR[0:K, _CHUNK:], dram_r[:, _CHUNK:])

            V = nc.vector
            S = nc.scalar

            RS = rpool.tile([P, NH * MT], f32, tag="RS", name="RS")
            RD = rpool.tile([P, 8], f16, tag="RD", name="RD")
            CR = rpool.tile([P, n_pts], f16, tag="CR", name="CR")

            for h in range(NH):
                cr_h = CR[:, h * _CHUNK:(h + 1) * _CHUNK]
                for blk in range(NBLK):
                    # exp'd units j=1..7 -> ring0 (j1-4), ring1 (j5-7);
                    # direct unit at j==0 so its VectorE drain runs at
                    # block start, overlapped by the next exps.
                    ring0 = ringpool.tile([P, 4, _CHUNK], bf16,
                                          tag="ring0", name="ring0")
                    ring1 = ringpool.tile([P, 3, _CHUNK], bf16,
                                          tag="ring1", name="ring1")
                    C2a = spool.tile([P, 2, _CHUNK], bf16, tag="C2a")
                    C2b = spool.tile([P, _CHUNK], bf16, tag="C2b")
                    CCs = ringpool.tile([P, _CHUNK], bf16, tag="CCs",
                                        name="CCs")
                    first = h == 0 and blk == 0
                    for j in range(8):
                        m = blk * 8 + j
                        u = h * MT + m
                        ps = ppool.tile([P, 64, 32], f32, tag="ps")
                        for q in range(4):
                            # row group rotates per 512-col matmul so
                            # LDWEIGHTS overlaps in-flight MATMULs.  In
                            # the very first block each unit pins one
                            # group so unit j depends only on the j-th
                            # prologue DMA pair.
                            g = j if first and j < 4 else q
                            col = _CHUNK * h + 512 * q
                            nc.tensor.matmul(
                                ps[:, 16 * q:16 * (q + 1), :],
                                sbW[32 * g:32 * g + K, m * P:(m + 1) * P],
                                sbR[32 * g:32 * g + K, col:col + 512],
                                start=True, stop=True,
                                tile_position=(32 * g, 0),
                            )
                        if j == 0:
                            # direct unit: VectorE drains PSUM.  Rows:
                            # 32:1 reduce then 64:1 (exact min, f16).
                            # Cols: mixed f32/f16 TT min into the raw
                            # per-half accumulator (copy to init).
                            diridx = h * 4 + blk
                            rd1 = spool.tile([P, 64], f16, tag="rd1")
                            V.tensor_reduce(rd1[:], ps[:], axis=AX, op=MIN)
                            if blk == 0:
                                V.tensor_copy(cr_h, ps[:, :, :])
                            else:
                                V.tensor_tensor(cr_h, ps[:, :, :], cr_h,
                                                op=MIN)
                            V.tensor_reduce(RD[:, diridx:diridx + 1],
                                            rd1[:], axis=AX, op=MIN)
                        else:
                            ring, jj = (ring0, j - 1) if j < 5 else (
                                ring1, j - 5)
                            S.activation(ring[:, jj, :], ps[:, :, :], EXP,
                                         bias=bv[:], scale=_SCALE,
                                         accum_out=RS[:, u:u + 1])
                        if j == 4:
                            V.tensor_tensor(C2a[:], ring0[:, 0:4:2, :],
                                            ring0[:, 1:4:2, :], op=MAX)
                        elif j == 6:
                            V.tensor_tensor(C2b[:], ring1[:, 0, :],
                                            ring1[:, 1, :], op=MAX)
                    # block tail: 7-tile max tree -> CCs, shipped per
                    # block (host maxes the 8 slabs x 128 partitions)
                    C4a = spool.tile([P, _CHUNK], bf16, tag="C4a")
                    C4b = spool.tile([P, _CHUNK], bf16, tag="C4b")
                    V.tensor_tensor(C4a[:], C2a[:, 0, :], C2a[:, 1, :],
                                    op=MAX)
                    V.tensor_tensor(C4b[:], C2b[:], ring1[:, 2, :],
                                    op=MAX)
                    V.tensor_tensor(CCs[:], C4a[:], C4b[:], op=MAX)
                    slab = (h * NBLK + blk) * _CHUNK
                    nc.sync.dma_start(dram_ce[:, slab:slab + _CHUNK],
                                      CCs[:])
                nc.sync.dma_start(dram_cr[:, h * _CHUNK:(h + 1) * _CHUNK],
                                  cr_h)
            nc.sync.dma_start(dram_rs[:], RS[:])
            nc.sync.dma_start(dram_rd[:], RD[:])

    nc.compile()
    return nc


def _get_program(n_pts=_N):
    if n_pts not in _prog_cache:
        _prog_cache[n_pts] = _build_program(n_pts)
    return _prog_cache[n_pts]


def _split_bf16(x32):
    """x32 fp32 -> (hi, lo) bf16 with hi+lo ~= x to ~2^-18 rel."""
    hi = x32.astype(_BF16)
    lo = (x32 - hi.astype(np.float32)).astype(_BF16)
    return hi, lo


def _split3(x64):
    """fp64 vector -> 3 bf16 terms summing to x to ~2^-27 rel."""
    t0 = x64.astype(_BF16)
    r = x64 - t0.astype(np.float64)
    t1 = r.astype(_BF16)
    r2 = r - t1.astype(np.float64)
    t2 = r2.astype(_BF16)
    return t0, t1, t2


def _features(q, c, n_pts):
    """Feature tensors for the distance matmul.

    q: query points  [3, N] fp32; c: candidate points [3, N] fp32.
    Returns (W [18, N] bf16, R [18, N] bf16) with
      (W.T @ R)[i, j] ~= |q~_i - c~_j|^2
    with ~ the bf16-split (hi+lo) values, exact to ~2e-6.
    """
    q_hi, q_lo = _split_bf16(q)
    c_hi, c_lo = _split_bf16(c)
    q_t = q_hi.astype(np.float32) + q_lo.astype(np.float32)
    c_t = c_hi.astype(np.float32) + c_lo.astype(np.float32)

    U = (c_t.astype(np.float64) ** 2).sum(axis=0)   # candidate norms
    u0, u1, u2 = _split3(U)
    V = (q_t.astype(np.float64) ** 2).sum(axis=0)   # query norms
    v0, v1, v2 = _split3(V)

    m2q_hi = (-2.0 * q_hi.astype(np.float32)).astype(_BF16)
    m2q_lo = (-2.0 * q_lo.astype(np.float32)).astype(_BF16)
    ones = np.ones(n_pts, dtype=_BF16)

    Wg = np.concatenate([
        m2q_hi, m2q_hi, m2q_lo, m2q_lo,
        np.stack([ones, ones, ones]),
        np.stack([v0, v1, v2]),
    ], axis=0).astype(_BF16)              # [18, N]
    Rg = np.concatenate([
        c_hi, c_lo, c_hi, c_lo,
        np.stack([u0, u1, u2]),
        np.stack([ones, ones, ones]),
    ], axis=0).astype(_BF16)              # [18, N]

    return Wg, Rg


def _host_prep(srcs_b, tgts_b):
    """Features + global softmin bias beta for one batch."""
    W, R = _features(srcs_b, tgts_b, _N)
    s = srcs_b.astype(np.float64)          # [3, N]
    t = tgts_b.astype(np.float64)
    rs = (s * s).sum(0)                    # |s_i|^2
    rt = (t * t).sum(0)
    # subsample estimate of per-row mins (stride 4 -> 1024 targets)
    tsub = t[:, ::4]
    cross = s.T @ tsub                     # [N, N/4]
    psub = rs[:, None] + rt[None, ::4] - 2.0 * cross
    bi = psub.min(axis=1)
    beta = float(bi.min())
    bvec = np.full((_P, 1), beta / _T, dtype=np.float32)
    return W, R, bvec, beta, rs, rt


def _host_post(res_b, beta, rs, rt, srcs_b, tgts_b):
    """Recover row/col minima for one batch; exact host fallback for
    flushed/overflowed entries."""
    s = srcs_b.astype(np.float64)
    t = tgts_b.astype(np.float64)
    MT = _N // _P

    rs_acc = res_b["rs_acc"].astype(np.float64)   # [128, 64]
    rdir = res_b["rdir"].astype(np.float64)       # [128, 8]
    colE = res_b["colE"].astype(np.float64)       # [128, 8*2048]
    colR = res_b["colR"].astype(np.float64)       # [128, 4096]

    # --- rows ---
    rowvals = np.empty(_N)
    bad_rows = []
    for m in range(MT):
        sl = slice(m * _P, (m + 1) * _P)
        if m % 8 == 0:
            d = m // 8
            rowvals[sl] = np.minimum(rdir[:, d], rdir[:, 4 + d])
        else:
            Rsum = rs_acc[:, m] + rs_acc[:, MT + m]
            with np.errstate(divide="ignore", invalid="ignore"):
                v = beta - _T * np.log(Rsum)
            bad = ~np.isfinite(v) | (v > beta + _ROW_THR)
            v = np.where(bad, np.inf, v)
            rowvals[sl] = v
            if bad.any():
                bad_rows.extend(m * _P + np.nonzero(bad)[0])
    if bad_rows:
        idx = np.asarray(bad_rows)
        d2 = (rs[idx, None] + rt[None, :]
              - 2.0 * (s[:, idx].T @ t))
        rowvals[idx] = d2.min(axis=1)

    # --- cols ---
    # colE slabs: (h*4+blk)*2048 + c -> target column 2048h + c
    slabs = colE.reshape(128, 2, 4, _CHUNK)       # [p, h, blk, c]
    maxE = slabs.max(axis=(0, 2)).reshape(-1)     # [4096]
    with np.errstate(divide="ignore", invalid="ignore"):
        vE = beta - _T * np.log(maxE)
    vR = colR.min(axis=0)
    colvals = np.minimum(np.where(np.isfinite(vE), vE, np.inf), vR)
    bad_c = ~np.isfinite(colvals) | (colvals > beta + _COL_THR)
    if bad_c.any():
        jdx = np.nonzero(bad_c)[0]
        d2 = (rs[:, None] + rt[jdx][None, :]
              - 2.0 * (s.T @ t[:, jdx]))
        colvals[jdx] = d2.min(axis=0)

    return rowvals.mean() + colvals.mean()


def kernel(srcs, tgts):
    import concourse.bass_utils as bass_utils

    srcs = np.asarray(srcs, dtype=np.float32)
    tgts = np.asarray(tgts, dtype=np.float32)
    B = srcs.shape[0]
    assert srcs.shape == (B, _D, _N) and tgts.shape == (B, _D, _N)

    nc = _get_program()

    in_maps = []
    host_ctx = []
    for b in range(B):
        W, R, bvec, beta, rs, rt = _host_prep(srcs[b], tgts[b])
        in_maps.append({"w": W, "r": R, "bv": bvec})
        host_ctx.append((beta, rs, rt))

    res = None
    for attempt in range(3):
        try:
            res = bass_utils.run_bass_kernel_spmd(
                nc, in_maps, core_ids=list(range(_NCORES)),
                trace=TRACE, trace_cores=TRACE_CORES if TRACE else None,
            )
            break
        except Exception:
            # transient NRT/device hiccups have been observed; retry
            if attempt == 2:
                raise
            import time
            time.sleep(3.0)
    global LAST_RESULTS
    LAST_RESULTS = res

    total = 0.0
    for b in range(B):
        beta, rs, rt = host_ctx[b]
        total += _host_post(res.results[b], beta, rs, rt,
                            srcs[b], tgts[b])

    return np.float32(total / B)


# revision 6
# speedup vs baseline: 1.1495x; 1.0068x over previous
"""Chamfer-distance (CDLoss) Trainium2 Bass kernel — exp-remap softmin hybrid.

Problem: srcs, tgts [B=8, D=3, N=4096] fp32.
  P[b,i,j] = |s_i|^2 + |t_j|^2 - 2 s_i.t_j
  out = min(P, axis=1).mean() + min(P, axis=2).mean()   (scalar fp32)

Strategy (data-parallel over B across 8 NeuronCores, one batch per core):
  The 4096x4096 distance matrix is produced tile-by-tile on TensorE
  (bf16 hi/lo-split features, K=18, [128,2048]-column units, PSUM f32)
  exactly as the classical baseline.  The bottleneck engine used to be
  VectorE (both min-trees, ~147us); the redesign moves the row
  reduction into the Activation engine's PSUM drain:

  * Act applies E = exp((beta - P)/T) (bias=beta/T per-partition const,
    scale=-1/T) while draining PSUM -> SBUF bf16.  Cost identical to
    the plain cast (measured 1967ns/tile), and `accum_out` yields the
    per-partition row sums Sum_j E for free (+182ns/tile): the row
    softmin is beta - T*ln(sum), computed on host.  Row min-tree: gone.
  * The col direction stays a classical tree, but in E-space: E is a
    monotone remap of P, so col max of E == col min of P (exact to
    bf16 rounding).  VectorE pair-max tree over the 7 exp'd tiles per
    block -> CC -> D1 -> A; A [128,2048] per half is shipped to DRAM
    and the host does the final 128-partition max (no device transpose
    tail).
  * 8 "direct" units (m%8==7, both halves) skip Act entirely: VectorE
    drains their PSUM with a 32:1 tensor_reduce min (rows, exact f16)
    and a mixed f32/f16 TT min into a raw per-half col accumulator.
    This rebalances ~17us of Act work onto VectorE's slack.

  Numerics: T=1e-3.  exp underflow flushes far pairs (harmless for
  min); rows/cols whose true min exceeds beta+~85T can flush/overflow
  - they are detected on host (nonfinite / threshold) and recomputed
  exactly there (~160 rows + ~160 cols per batch, measured; host cost
  ~60 Mflops).  Simulated end-to-end rel err 3.3e-3 vs 2e-2 gate.

  Per-core outputs:
    rs_acc [128, 64] f32   row exp-sums per (h,m) unit (exp'd units)
    rdir   [128, 8]  f16   exact row mins of the direct units
    colE   [128,4096] bf16 per-half col-max-of-E partials (over 128
                           partitions each; host maxes + log-maps)
    colR   [128,4096] f16  per-half raw col mins over direct units
"""

import numpy as np
import ml_dtypes

_BF16 = ml_dtypes.bfloat16

# Problem geometry (hardcoded per contest contract).
_B = 8
_D = 3
_N = 4096
_P = 128              # partitions / sources per m-tile
_K = 18               # feature rows (see _features)
_NCORES = 8
_CHUNK = 2048         # PSUM chunk columns (4 banks) == column half

_T = 1.0e-3           # softmin temperature
_SCALE = -1.0 / _T
_ROW_THR = 85.0 * _T  # host fallback threshold above beta
_COL_THR = 80.0 * _T

_prog_cache = {}

# test-harness knobs (the grading harness just calls kernel() and never
# touches these; default is the fast no-trace path)
TRACE = False
TRACE_CORES = [0]
LAST_RESULTS = None


def _build_program(n_pts=_N):
    import concourse.mybir as mybir
    import concourse.tile as tile
    from concourse import bacc

    P = _P
    MT = n_pts // P              # 32 m-tiles of 128 sources
    K = _K
    NH = n_pts // _CHUNK         # 2 column halves
    NBLK = MT // 8               # 4 eight-unit blocks per half
    f32 = mybir.dt.float32
    f16 = mybir.dt.float16
    bf16 = mybir.dt.bfloat16
    MIN = mybir.AluOpType.min
    MAX = mybir.AluOpType.max
    AX = mybir.AxisListType.X
    EXP = mybir.ActivationFunctionType.Exp

    nc = bacc.Bacc("TRN2", target_bir_lowering=False, debug=False,
                   num_devices=_NCORES)

    dram_w = nc.dram_tensor("w", [K, n_pts], bf16, kind="ExternalInput")
    dram_r = nc.dram_tensor("r", [K, n_pts], bf16, kind="ExternalInput")
    dram_bv = nc.dram_tensor("bv", [P, 1], f32, kind="ExternalInput")
    dram_rs = nc.dram_tensor("rs_acc", [P, NH * MT], f32,
                             kind="ExternalOutput")
    dram_rd = nc.dram_tensor("rdir", [P, 8], f16, kind="ExternalOutput")
    dram_ce = nc.dram_tensor("colE", [P, NH * NBLK * _CHUNK], bf16,
                             kind="ExternalOutput")
    dram_cr = nc.dram_tensor("colR", [P, n_pts], f16,
                             kind="ExternalOutput")

    with tile.TileContext(nc) as tc:
        with (
            tc.tile_pool(name="const", bufs=1) as cpool,
            tc.tile_pool(name="ring", bufs=2) as ringpool,
            tc.tile_pool(name="stage", bufs=1) as spool,
            tc.tile_pool(name="outs", bufs=1) as rpool,
            tc.tile_pool(name="psum", bufs=2, space="PSUM") as ppool,
        ):
            # Prologue: feature loads replicated into 4 PE row groups so
            # consecutive matmuls rotate groups (LDWEIGHTS overlaps
            # MATMUL).  Priority slices cover unit 0's operands so its
            # matmuls start early; bulk loads follow.  The Act queue is
            # kept DMA-free (Act is the bottleneck engine).
            sbW = cpool.tile([128, n_pts], bf16, tag="sbW", name="sbW")
            sbR = cpool.tile([128, n_pts], bf16, tag="sbR", name="sbR")
            bv = cpool.tile([P, 1], f32, tag="bv", name="bv")
            warm = cpool.tile([P, 1], f32, tag="warm", name="warm")
            nc.sync.dma_start(sbW[0:K, 0:2 * P], dram_w[:, 0:2 * P])
            nc.scalar.dma_start(sbR[0:K, 0:512], dram_r[:, 0:512])
            nc.scalar.dma_start(sbR[0:K, 512:_CHUNK],
                                dram_r[:, 512:_CHUNK])
            nc.sync.dma_start(bv[:], dram_bv[:])
            # warm the Act exp table at t=0 so the first real exp
            # doesn't eat the 1.3us ACT_TABLE_LOAD
            nc.vector.memset(warm[:], 0.0)
            nc.scalar.activation(warm[:], warm[:],
                                 mybir.ActivationFunctionType.Exp,
                                 bias=0.0, scale=1.0)
            nc.sync.dma_start(sbW[32 + 0:32 + K, :], dram_w[:])
            nc.sync.dma_start(sbR[32 + 0:32 + K, :], dram_r[:])
            for g in range(2, 4):
                nc.sync.dma_start(sbW[32 * g:32 * g + K, :], dram_w[:])
                nc.sync.dma_start(sbR[32 * g:32 * g + K, :], dram_r[:])
            nc.sync.dma_start(sbW[0:K, 2 * P:], dram_w[:, 2 * P:])
            nc.sync.dma_start(sbR[0:K, _CHUNK:], dram_r[:, _CHUNK:])

            V = nc.vector
            S = nc.scalar

            RS = rpool.tile([P, NH * MT], f32, tag="RS", name="RS")
            RD = rpool.tile([P, 8], f16, tag="RD", name="RD")
            CR = rpool.tile([P, n_pts], f16, tag="CR", name="CR")

            pending_tail = [None]

            def emit_tail():
                if pending_tail[0] is not None:
                    pending_tail[0]()
                    pending_tail[0] = None

            for h in range(NH):
                cr_h = CR[:, h * _CHUNK:(h + 1) * _CHUNK]
                for blk in range(NBLK):
                    # exp'd units j=1..7 -> ring0 (j1-4), ring1 (j5-7);
                    # direct unit at j==0 so its VectorE drain runs at
                    # block start, overlapped by the next exps.
                    ring0 = ringpool.tile([P, 4, _CHUNK], bf16,
                                          tag="ring0", name="ring0")
                    ring1 = ringpool.tile([P, 3, _CHUNK], bf16,
                                          tag="ring1", name="ring1")
                    C2a = ringpool.tile([P, 2, _CHUNK], bf16,
                                        tag="C2a", name="C2a")
                    C2b = ringpool.tile([P, _CHUNK], bf16, tag="C2b",
                                        name="C2b")
                    CCs = ringpool.tile([P, _CHUNK], bf16, tag="CCs",
                                        name="CCs")
                    first = h == 0 and blk == 0
                    for j in range(8):
                        m = blk * 8 + j
                        u = h * MT + m
                        ps = ppool.tile([P, 64, 32], f32, tag="ps")
                        for q in range(4):
                            # row group rotates per 512-col matmul so
                            # LDWEIGHTS overlaps in-flight MATMULs.  In
                            # the very first block each unit pins one
                            # group so unit j depends only on the j-th
                            # prologue DMA pair.
                            g = j if first and j < 4 else q
                            col = _CHUNK * h + 512 * q
                            nc.tensor.matmul(
                                ps[:, 16 * q:16 * (q + 1), :],
                                sbW[32 * g:32 * g + K, m * P:(m + 1) * P],
                                sbR[32 * g:32 * g + K, col:col + 512],
                                start=True, stop=True,
                                tile_position=(32 * g, 0),
                            )
                        if j == 0:
                            # direct unit: VectorE drains PSUM.  Rows:
                            # 32:1 reduce then 64:1 (exact min, f16).
                            # Cols: mixed f32/f16 TT min into the raw
                            # per-half accumulator (copy to init).
                            diridx = h * 4 + blk
                            rd1 = spool.tile([P, 64], f16, tag="rd1")
                            V.tensor_reduce(rd1[:], ps[:], axis=AX, op=MIN)
                            if blk == 0:
                                V.tensor_copy(cr_h, ps[:, :, :])
                            else:
                                V.tensor_tensor(cr_h, ps[:, :, :], cr_h,
                                                op=MIN)
                            V.tensor_reduce(RD[:, diridx:diridx + 1],
                                            rd1[:], axis=AX, op=MIN)
                            # previous block's col tail goes AFTER this
                            # drain in the DVE stream so the drain (and
                            # the PSUM buffer it holds) never queues
                            # behind ops that wait on Act
                            emit_tail()
                        else:
                            ring, jj = (ring0, j - 1) if j < 5 else (
                                ring1, j - 5)
                            S.activation(ring[:, jj, :], ps[:, :, :], EXP,
                                         bias=bv[:], scale=_SCALE,
                                         accum_out=RS[:, u:u + 1])
                        if j == 4:
                            V.tensor_tensor(C2a[:], ring0[:, 0:4:2, :],
                                            ring0[:, 1:4:2, :], op=MAX)
                        elif j == 6:
                            V.tensor_tensor(C2b[:], ring1[:, 0, :],
                                            ring1[:, 1, :], op=MAX)
                    # block tail: 7-tile max tree -> CCs, shipped
                    # per block; emission deferred into the next
                    # block's direct-drain slot (see emit_tail)
                    def make_tail(h=h, blk=blk, ring1=ring1, C2a=C2a,
                                  C2b=C2b, CCs=CCs):
                        def tail():
                            C4a = spool.tile([P, _CHUNK], bf16,
                                             tag="C4a")
                            C4b = spool.tile([P, _CHUNK], bf16,
                                             tag="C4b")
                            V.tensor_tensor(C4a[:], C2a[:, 0, :],
                                            C2a[:, 1, :], op=MAX)
                            V.tensor_tensor(C4b[:], C2b[:],
                                            ring1[:, 2, :], op=MAX)
                            V.tensor_tensor(CCs[:], C4a[:], C4b[:],
                                            op=MAX)
                            slab = (h * NBLK + blk) * _CHUNK
                            nc.sync.dma_start(
                                dram_ce[:, slab:slab + _CHUNK], CCs[:])
                        return tail
                    pending_tail[0] = make_tail()
                nc.sync.dma_start(dram_cr[:, h * _CHUNK:(h + 1) * _CHUNK],
                                  cr_h)
            emit_tail()
            nc.sync.dma_start(dram_rs[:], RS[:])
            nc.sync.dma_start(dram_rd[:], RD[:])

    nc.compile()
    return nc


def _get_program(n_pts=_N):
    if n_pts not in _prog_cache:
        _prog_cache[n_pts] = _build_program(n_pts)
    return _prog_cache[n_pts]


def _split_bf16(x32):
    """x32 fp32 -> (hi, lo) bf16 with hi+lo ~= x to ~2^-18 rel."""
    hi = x32.astype(_BF16)
    lo = (x32 - hi.astype(np.float32)).astype(_BF16)
    return hi, lo


def _split3(x64):
    """fp64 vector -> 3 bf16 terms summing to x to ~2^-27 rel."""
    t0 = x64.astype(_BF16)
    r = x64 - t0.astype(np.float64)
    t1 = r.astype(_BF16)
    r2 = r - t1.astype(np.float64)
    t2 = r2.astype(_BF16)
    return t0, t1, t2


def _features(q, c, n_pts):
    """Feature tensors for the distance matmul.

    q: query points  [3, N] fp32; c: candidate points [3, N] fp32.
    Returns (W [18, N] bf16, R [18, N] bf16) with
      (W.T @ R)[i, j] ~= |q~_i - c~_j|^2
    with ~ the bf16-split (hi+lo) values, exact to ~2e-6.
    """
    q_hi, q_lo = _split_bf16(q)
    c_hi, c_lo = _split_bf16(c)
    q_t = q_hi.astype(np.float32) + q_lo.astype(np.float32)
    c_t = c_hi.astype(np.float32) + c_lo.astype(np.float32)

    U = (c_t.astype(np.float64) ** 2).sum(axis=0)   # candidate norms
    u0, u1, u2 = _split3(U)
    V = (q_t.astype(np.float64) ** 2).sum(axis=0)   # query norms
    v0, v1, v2 = _split3(V)

    m2q_hi = (-2.0 * q_hi.astype(np.float32)).astype(_BF16)
    m2q_lo = (-2.0 * q_lo.astype(np.float32)).astype(_BF16)
    ones = np.ones(n_pts, dtype=_BF16)

    Wg = np.concatenate([
        m2q_hi, m2q_hi, m2q_lo, m2q_lo,
        np.stack([ones, ones, ones]),
        np.stack([v0, v1, v2]),
    ], axis=0).astype(_BF16)              # [18, N]
    Rg = np.concatenate([
        c_hi, c_lo, c_hi, c_lo,
        np.stack([u0, u1, u2]),
        np.stack([ones, ones, ones]),
    ], axis=0).astype(_BF16)              # [18, N]

    return Wg, Rg


def _host_prep(srcs_b, tgts_b):
    """Features + global softmin bias beta for one batch."""
    W, R = _features(srcs_b, tgts_b, _N)
    s = srcs_b.astype(np.float64)          # [3, N]
    t = tgts_b.astype(np.float64)
    rs = (s * s).sum(0)                    # |s_i|^2
    rt = (t * t).sum(0)
    # subsample estimate of per-row mins (stride 4 -> 1024 targets)
    tsub = t[:, ::4]
    cross = s.T @ tsub                     # [N, N/4]
    psub = rs[:, None] + rt[None, ::4] - 2.0 * cross
    bi = psub.min(axis=1)
    beta = float(bi.min())
    bvec = np.full((_P, 1), beta / _T, dtype=np.float32)
    return W, R, bvec, beta, rs, rt


def _host_post(res_b, beta, rs, rt, srcs_b, tgts_b):
    """Recover row/col minima for one batch; exact host fallback for
    flushed/overflowed entries."""
    s = srcs_b.astype(np.float64)
    t = tgts_b.astype(np.float64)
    MT = _N // _P

    rs_acc = res_b["rs_acc"].astype(np.float64)   # [128, 64]
    rdir = res_b["rdir"].astype(np.float64)       # [128, 8]
    colE = res_b["colE"].astype(np.float64)       # [128, 8*2048]
    colR = res_b["colR"].astype(np.float64)       # [128, 4096]

    # --- rows ---
    rowvals = np.empty(_N)
    bad_rows = []
    for m in range(MT):
        sl = slice(m * _P, (m + 1) * _P)
        if m % 8 == 0:
            d = m // 8
            rowvals[sl] = np.minimum(rdir[:, d], rdir[:, 4 + d])
        else:
            Rsum = rs_acc[:, m] + rs_acc[:, MT + m]
            with np.errstate(divide="ignore", invalid="ignore"):
                v = beta - _T * np.log(Rsum)
            bad = ~np.isfinite(v) | (v > beta + _ROW_THR)
            v = np.where(bad, np.inf, v)
            rowvals[sl] = v
            if bad.any():
                bad_rows.extend(m * _P + np.nonzero(bad)[0])
    if bad_rows:
        idx = np.asarray(bad_rows)
        d2 = (rs[idx, None] + rt[None, :]
              - 2.0 * (s[:, idx].T @ t))
        rowvals[idx] = d2.min(axis=1)

    # --- cols ---
    # colE slabs: (h*4+blk)*2048 + c -> target column 2048h + c
    slabs = colE.reshape(128, 2, 4, _CHUNK)       # [p, h, blk, c]
    maxE = slabs.max(axis=(0, 2)).reshape(-1)     # [4096]
    with np.errstate(divide="ignore", invalid="ignore"):
        vE = beta - _T * np.log(maxE)
    vR = colR.min(axis=0)
    colvals = np.minimum(np.where(np.isfinite(vE), vE, np.inf), vR)
    bad_c = ~np.isfinite(colvals) | (colvals > beta + _COL_THR)
    if bad_c.any():
        jdx = np.nonzero(bad_c)[0]
        d2 = (rs[:, None] + rt[jdx][None, :]
              - 2.0 * (s.T @ t[:, jdx]))
        colvals[jdx] = d2.min(axis=0)

    return rowvals.mean() + colvals.mean()


def kernel(srcs, tgts):
    import concourse.bass_utils as bass_utils

    srcs = np.asarray(srcs, dtype=np.float32)
    tgts = np.asarray(tgts, dtype=np.float32)
    B = srcs.shape[0]
    assert srcs.shape == (B, _D, _N) and tgts.shape == (B, _D, _N)

    nc = _get_program()

    in_maps = []
    host_ctx = []
    for b in range(B):
        W, R, bvec, beta, rs, rt = _host_prep(srcs[b], tgts[b])
        in_maps.append({"w": W, "r": R, "bv": bvec})
        host_ctx.append((beta, rs, rt))

    res = None
    for attempt in range(3):
        try:
            res = bass_utils.run_bass_kernel_spmd(
                nc, in_maps, core_ids=list(range(_NCORES)),
                trace=TRACE, trace_cores=TRACE_CORES if TRACE else None,
            )
            break
        except Exception:
            # transient NRT/device hiccups have been observed; retry
            if attempt == 2:
                raise
            import time
            time.sleep(3.0)
    global LAST_RESULTS
    LAST_RESULTS = res

    total = 0.0
    for b in range(B):
        beta, rs, rt = host_ctx[b]
        total += _host_post(res.results[b], beta, rs, rt,
                            srcs[b], tgts[b])

    return np.float32(total / B)
